# revision 23
# baseline (speedup 1.0000x reference)
"""Trainium2 Bass kernel for nn_DiscretizedManifoldTransformer.

Self-contained: takes FULL inputs (x [2,2048,768] f32 + params list of 4 layer
dicts), shards tokens across 8 NeuronCores, runs a single SPMD Bass program
(4 transformer blocks: 2x chunked card-passing layers, MLP, residual VQ), and
returns (x_out, ql, el) matching the jax reference.

Sharding: token-parallel. B*T = 4096 tokens -> 512 tokens (= 2 seq chunks of
256) per core. The only cross-core dependency is the carry prefix-scan over
chunk sums in each card layer; handled with a tiny (16x768 bf16) AllGather
followed by a per-core selection matmul.

Layouts (per core):
  T-major packed tile: [128, NT*F]   token-tile i in cols [i*F,(i+1)*F)
  C-major packed tile: [128, KC*512] channel-chunk k in cols [k*512,(k+1)*512)
Residual x stays resident in SBUF (f32). Matmul inputs bf16 (host-cast
weights); PSUM f32. sigmoid(g) is computed as 0.5*(tanh(g/2)+1) with the 0.5
folded algebraically downstream (cumsum runs at 2x scale; the cards LayerNorm
absorbs it exactly via a 4x eps). rsqrt = exponent-bits seed + ACT exp + 2
Newton steps (single ACT table set for the whole kernel). ql/el use
mean((q-res)^2) = ||res_next||^2/n with ||res||^2 tracked by r2 -= max_score.
"""

import sys

for _p in ("/opt/trn_rl_repo",):
    if _p not in sys.path:
        sys.path.insert(0, _p)

from contextlib import ExitStack

import numpy as np
import ml_dtypes

import concourse.bass as bass
import concourse.bacc as bacc
import concourse.mybir as mybir
import concourse.tile as tile
from concourse.bass_utils import run_bass_kernel_spmd
from concourse.masks import make_identity

F32 = mybir.dt.float32
BF16 = mybir.dt.bfloat16
I32 = mybir.dt.int32
U32 = mybir.dt.uint32
AF = mybir.ActivationFunctionType
ALU = mybir.AluOpType
AXX = mybir.AxisListType.X

B, T, C, H, S = 2, 2048, 768, 12, 256
D, N_CHUNKS = C // H, T // S          # 64, 8
FF, VL, NCODES, NLAYERS = 4 * C, 4, 1024, 4
EPS = 1e-5
NCORES = 8
TPC = B * T // NCORES                 # 512 tokens/core
NT = TPC // 128                       # 4 token tiles
KC = C // 128                         # 6
KF = FF // 128                        # 24
CHUNKS_PC = TPC // S                  # 2
LN2 = float(np.log(2.0))

bf16 = ml_dtypes.bfloat16

# feature flags (HW bring-up bisects)
USE_GATHER_ADD = False  # compute_op=add on indirect DMA wedges TRN2 (NRT_EXEC_UNIT_UNRECOVERABLE)


class Prog:
    def __init__(self, nc, tc, stats, scratch, consts):
        self.nc = nc
        self.tc = tc
        self.stats = stats
        self.scratch = scratch
        self.c = consts

    # -- tiny-op helpers ---------------------------------------------------

    def rsqrt(self, out_ap, in_ap, tag):
        """out = 1/sqrt(in), f32, small tiles. bits-seed + exp + 2 Newton."""
        nc = self.nc
        P_, n = in_ap.shape[0], in_ap.shape[1]
        t0 = self.stats.tile([P_, n], F32, name=f"{tag}_t0", tag=f"{tag}_t0")
        t1 = self.stats.tile([P_, n], F32, name=f"{tag}_t1", tag=f"{tag}_t1")
        nc.vector.tensor_copy(t0[:], in_ap.bitcast(I32))
        nc.vector.tensor_scalar(out=t0[:], in0=t0[:], scalar1=LN2 * (2.0 ** -23),
                                scalar2=127.0 * LN2, op0=ALU.mult, op1=ALU.subtract)
        nc.scalar.activation(out_ap, t0[:], AF.Exp, scale=-0.5)
        for _ in range(2):
            nc.vector.tensor_tensor(t1[:], out_ap, out_ap, op=ALU.mult)
            nc.vector.tensor_tensor(t1[:], t1[:], in_ap, op=ALU.mult)
            nc.vector.tensor_scalar(out=t1[:], in0=t1[:], scalar1=-0.5, scalar2=1.5,
                                    op0=ALU.mult, op1=ALU.add)
            nc.vector.tensor_tensor(out_ap, out_ap, t1[:], op=ALU.mult)

    def ln_stats(self, ssum_ap, ssq_ap, nelem, tag, eps_scale=1.0, want_r2=False,
                 want_mur=False):
        """mu = ssum/nelem, var = ssq/nelem - mu^2, r = rsqrt(var + eps_scale*EPS),
        mur = mu*r. Optional r2 = nelem*var*r^2. Returns (mu, r, mur, r2)."""
        nc = self.nc
        P_, n = ssum_ap.shape[0], ssum_ap.shape[1]
        st = self.stats
        mu = st.tile([P_, n], F32, name=f"{tag}_mu", tag=f"{tag}_mu")
        var = st.tile([P_, n], F32, name=f"{tag}_var", tag=f"{tag}_var")
        vpe = st.tile([P_, n], F32, name=f"{tag}_vpe", tag=f"{tag}_vpe")
        r = st.tile([P_, n], F32, name=f"{tag}_r", tag=f"{tag}_r")
        mur = st.tile([P_, n], F32, name=f"{tag}_mur", tag=f"{tag}_mur")
        inv = 1.0 / float(nelem)
        nc.vector.tensor_scalar(out=mu[:], in0=ssum_ap, scalar1=inv, scalar2=None, op0=ALU.mult)
        nc.vector.tensor_scalar(out=var[:], in0=ssq_ap, scalar1=inv, scalar2=None, op0=ALU.mult)
        nc.vector.tensor_tensor(vpe[:], mu[:], mu[:], op=ALU.mult)
        nc.vector.tensor_tensor(var[:], var[:], vpe[:], op=ALU.subtract)
        nc.vector.tensor_scalar(out=vpe[:], in0=var[:], scalar1=float(eps_scale) * EPS,
                                scalar2=None, op0=ALU.add)
        self.rsqrt(r[:], vpe[:], f"{tag}_rs")
        if want_mur:
            nc.vector.tensor_tensor(mur[:], mu[:], r[:], op=ALU.mult)
        r2 = None
        if want_r2:
            r2 = st.tile([P_, n], F32, name=f"{tag}_r2", tag=f"{tag}_r2")
            nc.vector.tensor_tensor(r2[:], r[:], r[:], op=ALU.mult)
            nc.vector.tensor_tensor(r2[:], r2[:], var[:], op=ALU.mult)
            nc.vector.tensor_scalar(out=r2[:], in0=r2[:], scalar1=float(nelem),
                                    scalar2=None, op0=ALU.mult)
        return mu, r, mur, r2

    def sums_of(self, x_fn, tag):
        """(sum, sumsq) [128, NT] over C for token tiles x_fn(i) [128, C] (SBUF f32)."""
        nc = self.nc
        ssum = self.stats.tile([128, NT], F32, name=f"{tag}_ssum", tag=f"{tag}_ssum")
        ssq = self.stats.tile([128, NT], F32, name=f"{tag}_ssq", tag=f"{tag}_ssq")
        for i in range(NT):
            xi = x_fn(i)
            nc.vector.tensor_reduce(out=ssum[:, i:i + 1], in_=xi, op=ALU.add, axis=AXX)
            sc = self.scratch.tile([128, C], BF16, name=f"{tag}_sc{i}", tag="lnsq_scratch")
            nc.vector.scalar_tensor_tensor(out=sc[:], in0=xi, scalar=0.0, in1=xi,
                                           op0=ALU.add, op1=ALU.mult,
                                           accum_out=ssq[:, i:i + 1])
        return ssum, ssq

    def ln_apply(self, out_fn, x_fn, mu, r, out_dtype_note=None):
        """out_i = (x_i - mu_i) * r_i for NT tiles."""
        nc = self.nc
        for i in range(NT):
            nc.vector.tensor_scalar(out=out_fn(i), in0=x_fn(i),
                                    scalar1=mu[:, i:i + 1], scalar2=r[:, i:i + 1],
                                    op0=ALU.subtract, op1=ALU.mult)

    def transpose_to_cmajor(self, dst_packed, src_fn, psum_pool, dtype, tag):
        """dst [128, KC*TPC] bf16 C-major <- src_fn(i) [128, C] T-major tiles."""
        nc = self.nc
        ident = self.c["ident_f32"] if dtype == F32 else self.c["ident_bf16"]
        for k in range(KC):
            pt = psum_pool.tile([128, TPC], dtype, name=f"{tag}_pt{k}",
                                tag=f"tp_{'f' if dtype == F32 else 'b'}")
            for i in range(NT):
                nc.tensor.transpose(pt[:, i * 128:(i + 1) * 128],
                                    src_fn(i)[:, k * 128:(k + 1) * 128], ident[:])
            nc.any.tensor_copy(dst_packed[:, k * TPC:(k + 1) * TPC], pt[:])

    def formB(self, psum_tile, xT_packed, w_packed, m, cout, kc=KC, nmax=512):
        """psum[128 tok, cout] = sum_k xT_chunk(k,m).T @ w_chunk(k) ."""
        nc = self.nc
        for s0 in range(0, cout, nmax):
            w_ = min(nmax, cout - s0)
            for k in range(kc):
                nc.tensor.matmul(psum_tile[:, s0:s0 + w_],
                                 lhsT=xT_packed[:, k * TPC + m * 128: k * TPC + (m + 1) * 128],
                                 rhs=w_packed[:, k * cout + s0: k * cout + s0 + w_],
                                 start=(k == 0), stop=(k == kc - 1))


def host_prep(x, params):
    xf = np.ascontiguousarray(np.asarray(x, np.float32).reshape(B * T, C))
    per_core_x = [np.ascontiguousarray(xf[c * TPC:(c + 1) * TPC]) for c in range(NCORES)]

    lsu = np.triu(np.ones((128, 128), np.float32), 1).astype(bf16)   # [s', s]: s' < s
    lones = np.ones((128, 128), np.float32).astype(bf16)
    chsel = np.zeros((CHUNKS_PC, CHUNKS_PC * 128), np.float32)
    for ch in range(CHUNKS_PC):
        chsel[ch, ch * 128:(ch + 1) * 128] = 1.0
    chsel = chsel.astype(bf16)

    msels = []
    for c in range(NCORES):
        m = np.zeros((NCORES * CHUNKS_PC, CHUNKS_PC), np.float32)
        for j in range(CHUNKS_PC):
            fc = c * CHUNKS_PC + j
            b_, n_ = fc // N_CHUNKS, fc % N_CHUNKS
            for k in range(NCORES * CHUNKS_PC):
                if k // N_CHUNKS == b_ and k % N_CHUNKS < n_:
                    m[k, j] = 1.0
        msels.append(m.astype(bf16))

    layers = []
    bad = []
    for li, p in enumerate(params):
        g = {k: np.asarray(v) for k, v in p.items()}
        lay = {nm: np.ascontiguousarray(g[nm].astype(bf16))
               for nm in ("pW", "m1W", "m2W", "h2W")}
        lay["mgW"] = np.ascontiguousarray(
            np.concatenate([g["mW"], g["gW"]], axis=1).astype(bf16))
        h1 = g["h1W"].astype(np.float32)
        lay["h1lo"] = np.ascontiguousarray(np.vstack([h1[0:64], h1[0:64]]).astype(bf16))
        lay["h1hi"] = np.ascontiguousarray(np.vstack([h1[64:128], h1[64:128]]).astype(bf16))
        books = g["books"].astype(np.float32)
        lay["books"] = [np.ascontiguousarray(books[l].astype(bf16)) for l in range(VL)]
        lay["books32"] = [np.ascontiguousarray(books[l]) for l in range(VL)]
        lay["e2t"] = [np.ascontiguousarray((2.0 * books[l].T).astype(bf16)) for l in range(VL)]
        lay["negsq"] = [np.ascontiguousarray(
            (-(books[l] ** 2).sum(1))[None, :].astype(np.float32)) for l in range(VL)]
        lay["ha"] = float(np.asarray(g["ha"]))
        lay["ma"] = float(np.asarray(g["ma"]))
        for nm in ("mb", "gb", "h1b", "h2b", "pb", "m1b", "m2b", "carry_b", "card_b",
                   "lnW_b", "ln1_b", "ln2_b", "ln3_b"):
            if not np.allclose(g[nm], 0.0):
                bad.append((li, nm))
        for nm in ("carry_g", "card_g", "lnW_g", "ln1_g", "ln2_g", "ln3_g"):
            if not np.allclose(g[nm], 1.0):
                bad.append((li, nm))
        layers.append(lay)
    if bad:
        raise NotImplementedError(f"non-identity LN gains / biases: {bad}")
    return {"lsu": lsu, "lones": lones, "chsel": chsel}, per_core_x, msels, layers


def build_program(layers, repeat=1):
    nc = bacc.Bacc("TRN2", target_bir_lowering=False, debug=False,
                   num_devices=NCORES)

    x_in = nc.dram_tensor("x_in", [TPC, C], F32, kind="ExternalInput").ap()
    msel_in = nc.dram_tensor("msel_in", [NCORES * CHUNKS_PC, CHUNKS_PC], BF16,
                             kind="ExternalInput").ap()
    lsu_in = nc.dram_tensor("lsu_in", [128, 128], BF16, kind="ExternalInput").ap()
    lones_in = nc.dram_tensor("lones_in", [128, 128], BF16, kind="ExternalInput").ap()
    chsel_in = nc.dram_tensor("chsel_in", [CHUNKS_PC, CHUNKS_PC * 128], BF16,
                              kind="ExternalInput").ap()
    wd = {}
    for li in range(NLAYERS):
        for nm, shp in (("mgW", [C, 2 * C]), ("pW", [C, C]),
                        ("m1W", [C, FF]), ("m2W", [FF, C]),
                        ("h1lo", [2 * D, 2 * D]), ("h1hi", [2 * D, 2 * D]),
                        ("h2W", [2 * D, D])):
            wd[(li, nm)] = nc.dram_tensor(f"L{li}_{nm}", shp, BF16, kind="ExternalInput").ap()
        for l in range(VL):
            wd[(li, "book", l)] = nc.dram_tensor(f"L{li}_book{l}", [NCODES, C], BF16,
                                                 kind="ExternalInput").ap()
            if USE_GATHER_ADD:
                wd[(li, "book32", l)] = nc.dram_tensor(f"L{li}_book32_{l}", [NCODES, C], F32,
                                                       kind="ExternalInput").ap()
            wd[(li, "e2t", l)] = nc.dram_tensor(f"L{li}_e2t{l}", [C, NCODES], BF16,
                                                kind="ExternalInput").ap()
            wd[(li, "negsq", l)] = nc.dram_tensor(f"L{li}_negsq{l}", [1, NCODES], F32,
                                                  kind="ExternalInput").ap()
    y_out = nc.dram_tensor("y_out", [TPC, C], F32, kind="ExternalOutput").ap()
    ql_out = nc.dram_tensor("ql_out", [128, NT], F32, kind="ExternalOutput").ap()

    with tile.TileContext(nc) as tc, ExitStack() as top:
        const_pool = top.enter_context(tc.tile_pool(name="consts", bufs=1))
        resident = top.enter_context(tc.tile_pool(name="resident", bufs=1))
        stats = top.enter_context(tc.tile_pool(name="stats", bufs=1))
        scratch = top.enter_context(tc.tile_pool(name="scratch", bufs=3))
        dram_pool = top.enter_context(tc.tile_pool(name="drambounce", bufs=2, space="DRAM"))

        ident_f32 = const_pool.tile([128, 128], F32)
        ident_bf16 = const_pool.tile([128, 128], BF16)
        make_identity(nc, ident_f32[:])
        make_identity(nc, ident_bf16[:])
        lsu = const_pool.tile([128, 128], BF16)
        lones = const_pool.tile([128, 128], BF16)
        msel = const_pool.tile([NCORES * CHUNKS_PC, CHUNKS_PC], BF16)
        chsel = const_pool.tile([CHUNKS_PC, CHUNKS_PC * 128], BF16)
        nc.sync.dma_start(lsu[:], lsu_in[:])
        nc.sync.dma_start(lones[:], lones_in[:])
        nc.sync.dma_start(msel[:], msel_in[:])
        nc.sync.dma_start(chsel[:], chsel_in[:])

        x_res = resident.tile([128, NT * C], F32)
        ql_acc = resident.tile([128, NT], F32)
        r2 = resident.tile([128, NT], F32)

        consts = {"ident_f32": ident_f32, "ident_bf16": ident_bf16}
        pr = Prog(nc, tc, stats, scratch, consts)

        def xr(i):
            return x_res[:, i * C:(i + 1) * C]

        x3sum = None  # carried from MLP residual add into ln3

        for li_r in range(repeat * NLAYERS):
            li = li_r
            lay = layers[li_r % NLAYERS]
            if li_r % NLAYERS == 0:
                # (re)load residual + zero the ql accumulator; repeats>1 are
                # only used for marginal-time benchmarking
                nc.vector.memset(ql_acc[:], 0.0)
                for i in range(NT):
                    nc.sync.dma_start(x_res[:, i * C:(i + 1) * C],
                                      x_in[i * 128:(i + 1) * 128, :])
            layer_stack = ExitStack()
            pr.stats = layer_stack.enter_context(
                tc.tile_pool(name=f"st{li}", bufs=1))

            # ================= card deliberation (2 calls) =================
            with ExitStack() as st:
                wpool = st.enter_context(tc.tile_pool(name=f"cw{li}", bufs=1))
                mg_sb = wpool.tile([128, KC * 2 * C], BF16, name=f"mg{li}")
                pw_sb = wpool.tile([128, KC * C], BF16, name=f"pw{li}")
                h1lo_sb = wpool.tile([128, 2 * D], BF16, name=f"h1lo{li}")
                h1hi_sb = wpool.tile([128, 2 * D], BF16, name=f"h1hi{li}")
                h2_sb = wpool.tile([128, D], BF16, name=f"h2{li}")
                for k in range(KC):
                    nc.sync.dma_start(mg_sb[:, k * 2 * C:(k + 1) * 2 * C], wd[(li % NLAYERS, "mgW")][k * 128:(k + 1) * 128, :])
                    nc.sync.dma_start(pw_sb[:, k * C:(k + 1) * C], wd[(li % NLAYERS, "pW")][k * 128:(k + 1) * 128, :])
                nc.sync.dma_start(h1lo_sb[:], wd[(li % NLAYERS, "h1lo")][:])
                nc.sync.dma_start(h1hi_sb[:], wd[(li % NLAYERS, "h1hi")][:])
                nc.sync.dma_start(h2_sb[:], wd[(li % NLAYERS, "h2W")][:])

                apool = st.enter_context(tc.tile_pool(name=f"ca{li}", bufs=1))
                u_tile = apool.tile([128, NT * C], F32, name=f"u{li}")

                cur_src = x_res
                for call in range(2):
                    tg = f"{li}_{call}"
                    with ExitStack() as cs:
                        cpool = cs.enter_context(tc.tile_pool(name=f"c{tg}", bufs=1))

                        def cur(i, _s=cur_src):
                            return _s[:, i * C:(i + 1) * C]

                        # ---- t = ln1(cur); tT (C-major bf16)
                        ssum, ssq = pr.sums_of(cur, f"l1_{tg}")
                        mu, r, mur, _ = pr.ln_stats(ssum[:], ssq[:], C, f"l1s_{tg}")
                        t_f32 = cpool.tile([128, NT * C], F32, name=f"t{tg}")
                        pr.ln_apply(lambda i: t_f32[:, i * C:(i + 1) * C], cur, mu, r)
                        tT = cpool.tile([128, KC * TPC], BF16, name=f"tT{tg}")
                        with tc.tile_pool(name=f"ptp{tg}", bufs=2, space="PSUM") as pst:
                            pr.transpose_to_cmajor(tT[:], lambda i: t_f32[:, i * C:(i + 1) * C],
                                                   pst, F32, f"tT{tg}")

                        # ---- marks & gates -> gm (2x scale: (tanh+1)*marks)
                        gm = cpool.tile([128, NT * C], BF16, name=f"gm{tg}")
                        with tc.tile_pool(name=f"pmg{tg}", bufs=2, space="PSUM") as pmg:
                            for m in range(NT):
                                ps_mg = pmg.tile([128, 2 * C], F32, name=f"psmg{m}", tag="ps_mg")
                                pr.formB(ps_mg, tT[:], mg_sb[:], m, 2 * C)
                                gt_t = scratch.tile([128, C], BF16, name=f"gtt{m}", tag="gt_tanh")
                                nc.scalar.activation(gt_t[:], ps_mg[:, C:2 * C], AF.Tanh, scale=0.5)
                                nc.vector.scalar_tensor_tensor(
                                    out=gm[:, m * C:(m + 1) * C], in0=gt_t[:], scalar=1.0,
                                    in1=ps_mg[:, 0:C], op0=ALU.add, op1=ALU.mult)

                        # ---- chunk sums (2x scale) -> AllGather -> carry -> ncar
                        cs_sb = cpool.tile([32 * (CHUNKS_PC - 1) + 1, C], BF16, name=f"cssb{tg}")
                        with tc.tile_pool(name=f"pcs{tg}", bufs=1, space="PSUM") as pcs:
                            for ch in range(CHUNKS_PC):
                                ps_cs = pcs.tile([1, C], F32, name=f"pscs{ch}", tag=f"ps_cs{ch}")
                                for half in range(2):
                                    m = 2 * ch + half
                                    for s0 in range(0, C, 512):
                                        w_ = min(512, C - s0)
                                        nc.tensor.matmul(
                                            ps_cs[:, s0:s0 + w_], lhsT=lones[:, 0:1],
                                            rhs=gm[:, m * C + s0: m * C + s0 + w_],
                                            start=(half == 0), stop=(half == 1))
                                nc.scalar.copy(cs_sb[32 * ch:32 * ch + 1, :], ps_cs[:])

                        bounce_in = dram_pool.tile([CHUNKS_PC, C], BF16,
                                                   name=f"bin{tg}", tag="bounce_in")
                        bounce_out = dram_pool.tile([NCORES * CHUNKS_PC, C], BF16,
                                                    name=f"bout{tg}", tag="bounce_out")
                        for ch in range(CHUNKS_PC):
                            nc.sync.dma_start(bounce_in[ch:ch + 1, :],
                                              cs_sb[32 * ch:32 * ch + 1, :])
                        nc.gpsimd.collective_compute(
                            "AllGather", ALU.bypass, replica_groups=[list(range(NCORES))],
                            ins=[bounce_in[:].opt()], outs=[bounce_out[:].opt()])
                        gath = cpool.tile([NCORES * CHUNKS_PC, C], BF16, name=f"gath{tg}")
                        nc.sync.dma_start(gath[:], bounce_out[:])

                        ncar = cpool.tile([CHUNKS_PC, C], BF16, name=f"ncar{tg}")
                        with tc.tile_pool(name=f"pcy{tg}", bufs=1, space="PSUM") as pcy:
                            ps_cy = pcy.tile([CHUNKS_PC, C], F32, name="pscy", tag="ps_cy")
                            for s0 in range(0, C, 512):
                                w_ = min(512, C - s0)
                                nc.tensor.matmul(ps_cy[:, s0:s0 + w_], lhsT=msel[:],
                                                 rhs=gath[:, s0:s0 + w_], start=True, stop=True)
                            # exact carry = 0.5 * psum
                            carry_f = cpool.tile([CHUNKS_PC, C], F32, name=f"cyf{tg}")
                            nc.vector.tensor_scalar(out=carry_f[:], in0=ps_cy[:], scalar1=0.5,
                                                    scalar2=None, op0=ALU.mult)
                        csum = stats.tile([CHUNKS_PC, H], F32, name="cysum", tag=f"cy_s{tg}")
                        csq = stats.tile([CHUNKS_PC, H], F32, name="cysq", tag=f"cy_q{tg}")
                        sqt = scratch.tile([CHUNKS_PC, C], F32, name="cysq2", tag="cy_sq")
                        nc.vector.tensor_tensor(sqt[:], carry_f[:], carry_f[:], op=ALU.mult)
                        nc.vector.tensor_reduce(out=csum[:], in_=carry_f[:].rearrange(
                            "p (g s) -> p g s", s=D), op=ALU.add, axis=AXX)
                        nc.vector.tensor_reduce(out=csq[:], in_=sqt[:].rearrange(
                            "p (g s) -> p g s", s=D), op=ALU.add, axis=AXX)
                        cmu, cr, _, _ = pr.ln_stats(csum[:], csq[:], D, f"cy_{tg}")
                        # ncar2 = 2 * LN(carry)  (2x so it matches gm's 2x scale)
                        cr2 = stats.tile([CHUNKS_PC, H], F32, name="cr2", tag=f"cy_r2{tg}")
                        nc.vector.tensor_scalar(out=cr2[:], in0=cr[:], scalar1=2.0,
                                                scalar2=None, op0=ALU.mult)
                        cen = scratch.tile([CHUNKS_PC, C], F32, name="cycen", tag="cy_cen")
                        nc.vector.tensor_tensor(
                            cen[:].rearrange("p (g s) -> p g s", s=D),
                            carry_f[:].rearrange("p (g s) -> p g s", s=D),
                            cmu[:].broadcast_to([CHUNKS_PC, H, D]), op=ALU.subtract)
                        nc.vector.tensor_tensor(
                            ncar[:].rearrange("p (g s) -> p g s", s=D),
                            cen[:].rearrange("p (g s) -> p g s", s=D),
                            cr2[:].broadcast_to([CHUNKS_PC, H, D]), op=ALU.mult)

                        # ---- cards: cumsum matmuls issued before the gather
                        # lands (they only need gm); ncar row + LN_D after.
                        cards = cpool.tile([128, NT * C], BF16, name=f"cards{tg}")
                        with tc.tile_pool(name=f"pcd{tg}", bufs=2, space="PSUM") as pcd:
                            for ch in range(CHUNKS_PC):
                                ps_cds = []
                                for half in range(2):
                                    m = 2 * ch + half
                                    ps_cd = pcd.tile([128, C], F32, name=f"pscd{m}", tag=f"ps_cd{half}")
                                    for s0 in range(0, C, 512):
                                        w_ = min(512, C - s0)
                                        first = True
                                        if half == 1:
                                            nc.tensor.matmul(
                                                ps_cd[:, s0:s0 + w_], lhsT=lones[:],
                                                rhs=gm[:, (m - 1) * C + s0: (m - 1) * C + s0 + w_],
                                                start=True, stop=False)
                                            first = False
                                        nc.tensor.matmul(
                                            ps_cd[:, s0:s0 + w_], lhsT=lsu[:],
                                            rhs=gm[:, m * C + s0: m * C + s0 + w_],
                                            start=first, stop=False)
                                    ps_cds.append(ps_cd)
                                for half in range(2):
                                    m = 2 * ch + half
                                    ps_cd = ps_cds[half]
                                    for s0 in range(0, C, 512):
                                        w_ = min(512, C - s0)
                                        nc.tensor.matmul(
                                            ps_cd[:, s0:s0 + w_],
                                            lhsT=chsel[:, ch * 128:(ch + 1) * 128],
                                            rhs=ncar[:, s0:s0 + w_],
                                            start=False, stop=True)
                                    dsum = pr.stats.tile([128, H], F32, name=f"cdsum{m}", tag="cd_s")
                                    dsq = pr.stats.tile([128, H], F32, name=f"cdsq{m}", tag="cd_q")
                                    sq2 = scratch.tile([128, C], BF16, name=f"cdsq2{m}", tag="cd_sq")
                                    nc.scalar.activation(sq2[:], ps_cd[:], AF.Square)
                                    nc.vector.tensor_reduce(out=dsum[:], in_=ps_cd[:].rearrange(
                                        "p (g s) -> p g s", s=D), op=ALU.add, axis=AXX)
                                    nc.vector.tensor_reduce(out=dsq[:], in_=sq2[:].rearrange(
                                        "p (g s) -> p g s", s=D), op=ALU.add, axis=AXX)
                                    dmu, dr, _, _ = pr.ln_stats(dsum[:], dsq[:], D,
                                                                f"cd_{tg}_{m}", eps_scale=4.0)
                                    dr_bf = pr.stats.tile([128, H], BF16, name=f"drb{m}", tag="cd_rb")
                                    nc.vector.tensor_copy(dr_bf[:], dr[:])
                                    cen2 = scratch.tile([128, C], BF16, name=f"cdc{m}", tag="cd_cen")
                                    nc.vector.tensor_tensor(
                                        cen2[:].rearrange("p (g s) -> p g s", s=D),
                                        ps_cd[:].rearrange("p (g s) -> p g s", s=D),
                                        dmu[:].broadcast_to([128, H, D]), op=ALU.subtract)
                                    nc.vector.tensor_tensor(
                                        cards[:, m * C:(m + 1) * C].rearrange("p (g s) -> p g s", s=D),
                                        cen2[:].rearrange("p (g s) -> p g s", s=D),
                                        dr_bf[:].broadcast_to([128, H, D]), op=ALU.mult)

                        cardsT = cpool.tile([128, KC * TPC], BF16, name=f"cdT{tg}")
                        with tc.tile_pool(name=f"ptc{tg}", bufs=2, space="PSUM") as pst:
                            pr.transpose_to_cmajor(cardsT[:], lambda i: cards[:, i * C:(i + 1) * C],
                                                   pst, BF16, f"cdT{tg}")

                        # ---- per-head h1 + FEGE + h2 -> hoT (C-major bf16)
                        hoT = cpool.tile([128, KC * TPC], BF16, name=f"hoT{tg}")
                        ha = lay["ha"]
                        with tc.tile_pool(name=f"phh{tg}", bufs=3, space="PSUM") as phh:
                            for h in range(H):
                                k, rowh = divmod(h, 2)
                                rsl = slice(rowh * 64, rowh * 64 + 64)
                                ps_h = phh.tile([128, TPC], F32, name=f"psh{h}", tag="ps_h1")
                                nc.tensor.matmul(ps_h[:], lhsT=h1lo_sb[rsl, :],
                                                 rhs=tT[:, k * TPC:(k + 1) * TPC][rsl, :],
                                                 start=True, stop=False)
                                nc.tensor.matmul(ps_h[:], lhsT=h1hi_sb[rsl, :],
                                                 rhs=cardsT[:, k * TPC:(k + 1) * TPC][rsl, :],
                                                 start=False, stop=True)
                                sqh = scratch.tile([128, TPC], BF16, name=f"sqh{h}", tag="fg_sq")
                                nc.scalar.activation(sqh[:], ps_h[:], AF.Square)
                                eh = scratch.tile([128, TPC], BF16, name=f"eh{h}", tag="fg_e")
                                nc.scalar.activation(eh[:], sqh[:], AF.Exp, scale=-0.5)
                                nc.vector.tensor_scalar(out=eh[:], in0=eh[:], scalar1=ha,
                                                        scalar2=1.0, op0=ALU.mult, op1=ALU.add)
                                hf = scratch.tile([128, TPC], BF16, name=f"hf{h}", tag="fg_hf")
                                nc.vector.tensor_tensor(hf[:], ps_h[:], eh[:], op=ALU.mult)
                                ps_o = phh.tile([128, TPC], F32, name=f"pso{h}", tag="ps_h2",
                                                bufs=2) if rowh == 0 else ps_o_prev
                                tp = (0, 64) if rowh == 1 else None
                                nc.tensor.matmul(ps_o[rsl, :], lhsT=h2_sb[:], rhs=hf[:],
                                                 start=True, stop=True, tile_position=tp)
                                if rowh == 1:
                                    nc.any.tensor_copy(hoT[:, k * TPC:(k + 1) * TPC], ps_o[:])
                                ps_o_prev = ps_o

                        # ---- pW + lnW + residual: dst = x + t + LN(o)
                        dst = u_tile if call == 0 else x_res
                        o_f32 = cpool.tile([128, NT * C], F32, name=f"of{tg}")
                        osum = stats.tile([128, NT], F32, name="osum", tag=f"lnw_s{tg}")
                        osq = stats.tile([128, NT], F32, name="osq", tag=f"lnw_q{tg}")
                        with tc.tile_pool(name=f"ppw{tg}", bufs=3, space="PSUM") as ppw:
                            for m in range(NT):
                                ps_p = ppw.tile([128, C], F32, name=f"psp{m}", tag="ps_pw")
                                pr.formB(ps_p, hoT[:], pw_sb[:], m, C)
                                nc.scalar.activation(o_f32[:, m * C:(m + 1) * C], ps_p[:],
                                                     AF.Copy, accum_out=osum[:, m:m + 1])
                                junk = scratch.tile([128, C], BF16, name=f"jk{m}", tag="lnw_junk")
                                nc.scalar.activation(junk[:], ps_p[:], AF.Square,
                                                     accum_out=osq[:, m:m + 1])
                        omu, orr, omur, _ = pr.ln_stats(osum[:], osq[:], C, f"lnw_{tg}", want_mur=True)
                        for m in range(NT):
                            xa = scratch.tile([128, C], F32, name=f"xa{m}", tag="xa")
                            # w = o*r + t ; dst = (w - mu*r) + x
                            nc.vector.scalar_tensor_tensor(
                                out=xa[:], in0=o_f32[:, m * C:(m + 1) * C],
                                scalar=orr[:, m:m + 1], in1=t_f32[:, m * C:(m + 1) * C],
                                op0=ALU.mult, op1=ALU.add)
                            nc.vector.scalar_tensor_tensor(
                                out=dst[:, m * C:(m + 1) * C], in0=xa[:],
                                scalar=omur[:, m:m + 1], in1=x_res[:, m * C:(m + 1) * C],
                                op0=ALU.subtract, op1=ALU.add)
                    cur_src = u_tile

            # ================= MLP =================
            with ExitStack() as st:
                wpool = st.enter_context(tc.tile_pool(name=f"mw{li}", bufs=1))
                m1_sb = wpool.tile([128, KC * FF], BF16, name=f"m1{li}")
                m2_sb = wpool.tile([128, KF * C], BF16, name=f"m2{li}")
                for k in range(KC):
                    nc.sync.dma_start(m1_sb[:, k * FF:(k + 1) * FF],
                                      wd[(li % NLAYERS, "m1W")][k * 128:(k + 1) * 128, :])
                for k in range(KF):
                    nc.sync.dma_start(m2_sb[:, k * C:(k + 1) * C],
                                      wd[(li % NLAYERS, "m2W")][k * 128:(k + 1) * 128, :])
                mpool = st.enter_context(tc.tile_pool(name=f"ma{li}", bufs=1))

                ssum, ssq = pr.sums_of(xr, f"l2_{li}")
                mu, r, _, _ = pr.ln_stats(ssum[:], ssq[:], C, f"l2s_{li}")
                xn2 = mpool.tile([128, NT * C], BF16, name=f"xn2_{li}")
                pr.ln_apply(lambda i: xn2[:, i * C:(i + 1) * C], xr, mu, r)
                xn2T = mpool.tile([128, KC * TPC], BF16, name=f"xn2T_{li}")
                with tc.tile_pool(name=f"ptm{li}", bufs=2, space="PSUM") as pst:
                    pr.transpose_to_cmajor(xn2T[:], lambda i: xn2[:, i * C:(i + 1) * C],
                                           pst, BF16, f"xn2T{li}")

                hmlp = mpool.tile([128, KF * TPC], BF16, name=f"hm_{li}")
                ma = lay["ma"]
                with tc.tile_pool(name=f"pm1{li}", bufs=4, space="PSUM") as pm1:
                    for mo in range(KF):
                        ps_m = pm1.tile([128, TPC], F32, name=f"psm{mo}", tag="ps_m1")
                        for k in range(KC):
                            nc.tensor.matmul(
                                ps_m[:], lhsT=m1_sb[:, k * FF + mo * 128: k * FF + (mo + 1) * 128],
                                rhs=xn2T[:, k * TPC:(k + 1) * TPC],
                                start=(k == 0), stop=(k == KC - 1))
                        sqh = scratch.tile([128, TPC], BF16, name=f"msq{mo}", tag="fg_sq")
                        nc.scalar.activation(sqh[:], ps_m[:], AF.Square)
                        eh = scratch.tile([128, TPC], BF16, name=f"me{mo}", tag="fg_e")
                        nc.scalar.activation(eh[:], sqh[:], AF.Exp, scale=-0.5)
                        nc.vector.tensor_scalar(out=eh[:], in0=eh[:], scalar1=ma,
                                                scalar2=1.0, op0=ALU.mult, op1=ALU.add)
                        nc.vector.tensor_tensor(hmlp[:, mo * TPC:(mo + 1) * TPC], ps_m[:],
                                                eh[:], op=ALU.mult)

                x3sum = stats.tile([128, NT], F32, name="x3sum", tag=f"x3_s{li}")
                with tc.tile_pool(name=f"pm2{li}", bufs=2, space="PSUM") as pm2:
                    for m in range(NT):
                        ps_o = pm2.tile([128, C], F32, name=f"psmo{m}", tag="ps_m2")
                        pr.formB(ps_o, hmlp[:], m2_sb[:], m, C, kc=KF)
                        nc.vector.scalar_tensor_tensor(
                            out=x_res[:, m * C:(m + 1) * C], in0=ps_o[:], scalar=1.0,
                            in1=x_res[:, m * C:(m + 1) * C], op0=ALU.mult, op1=ALU.add,
                            accum_out=x3sum[:, m:m + 1])

            # ================= RVQ =================
            with ExitStack() as st:
                rpool = st.enter_context(tc.tile_pool(name=f"rv{li}", bufs=1))
                epool = st.enter_context(tc.tile_pool(name=f"re{li}", bufs=2))

                ssq3 = stats.tile([128, NT], F32, name="ssq3", tag=f"x3_q{li}")
                for i in range(NT):
                    sc = scratch.tile([128, C], BF16, name=f"x3sc{i}", tag="lnsq_scratch")
                    nc.vector.scalar_tensor_tensor(out=sc[:], in0=xr(i), scalar=0.0, in1=xr(i),
                                                   op0=ALU.add, op1=ALU.mult,
                                                   accum_out=ssq3[:, i:i + 1])
                mu3, r3, _, r2_0 = pr.ln_stats(x3sum[:], ssq3[:], C, f"l3s_{li}", want_r2=True)
                nc.vector.tensor_copy(r2[:], r2_0[:])
                xn3 = rpool.tile([128, NT * C], BF16, name=f"xn3_{li}")
                pr.ln_apply(lambda i: xn3[:, i * C:(i + 1) * C], xr, mu3, r3)
                resT = rpool.tile([128, KC * TPC], BF16, name=f"resT_{li}")
                with tc.tile_pool(name=f"ptr{li}", bufs=2, space="PSUM") as pst:
                    pr.transpose_to_cmajor(resT[:], lambda i: xn3[:, i * C:(i + 1) * C],
                                           pst, BF16, f"resT{li}")

                mv = stats.tile([128, NT], F32, name="mv", tag=f"mv{li}")
                for lv in range(VL):
                    e2t_sb = epool.tile([128, KC * NCODES], BF16, name=f"e2t{lv}", tag="e2t")
                    for k in range(KC):
                        nc.sync.dma_start(e2t_sb[:, k * NCODES:(k + 1) * NCODES],
                                          wd[(li % NLAYERS, "e2t", lv)][k * 128:(k + 1) * 128, :])
                    nsq_sb = epool.tile([1, NCODES], F32, name=f"nsq{lv}", tag="nsq")
                    nc.sync.dma_start(nsq_sb[:], wd[(li % NLAYERS, "negsq", lv)][:])
                    nsq_bf = epool.tile([1, NCODES], BF16, name=f"nsqb{lv}", tag="nsqb")
                    nc.vector.tensor_copy(nsq_bf[:], nsq_sb[:])

                    idx_t = rpool.tile([128, NT * 8], U32, name=f"idx{lv}", tag="idx")
                    q_sb = rpool.tile([128, NT * C], BF16, name=f"q{lv}", tag="q_sb")
                    with tc.tile_pool(name=f"psc{li}_{lv}", bufs=3, space="PSUM") as psc:
                        for m in range(NT):
                            ps_s = psc.tile([128, NCODES], F32, name=f"pss{m}", tag="ps_sc")
                            for s0 in range(0, NCODES, 512):
                                for k in range(KC):
                                    nc.tensor.matmul(
                                        ps_s[:, s0:s0 + 512],
                                        lhsT=resT[:, k * TPC + m * 128: k * TPC + (m + 1) * 128],
                                        rhs=e2t_sb[:, k * NCODES + s0: k * NCODES + s0 + 512],
                                        start=(k == 0), stop=False)
                                nc.tensor.matmul(ps_s[:, s0:s0 + 512], lhsT=lones[0:1, :],
                                                 rhs=nsq_bf[:, s0:s0 + 512],
                                                 start=False, stop=True)
                            mx8 = stats.tile([128, 8], F32, name=f"mx8_{m}", tag=f"mx8_{m}")
                            nc.vector.max(mx8[:], ps_s[:])
                            nc.vector.tensor_copy(mv[:, m:m + 1], mx8[:, 0:1])
                            nc.vector.max_index(idx_t[:, m * 8:(m + 1) * 8], mx8[:], ps_s[:])
                            if USE_GATHER_ADD:
                                nc.gpsimd.indirect_dma_start(
                                    out=x_res[:, m * C:(m + 1) * C], out_offset=None,
                                    in_=wd[(li % NLAYERS, "book32", lv)][:],
                                    in_offset=bass.IndirectOffsetOnAxis(
                                        ap=idx_t[:, m * 8:m * 8 + 1].bitcast(I32), axis=0),
                                    compute_op=ALU.add)
                                if lv < VL - 1:
                                    nc.gpsimd.indirect_dma_start(
                                        out=q_sb[:, m * C:(m + 1) * C], out_offset=None,
                                        in_=wd[(li % NLAYERS, "book", lv)][:],
                                        in_offset=bass.IndirectOffsetOnAxis(
                                            ap=idx_t[:, m * 8:m * 8 + 1].bitcast(I32), axis=0))
                            else:
                                nc.gpsimd.indirect_dma_start(
                                    out=q_sb[:, m * C:(m + 1) * C], out_offset=None,
                                    in_=wd[(li % NLAYERS, "book", lv)][:],
                                    in_offset=bass.IndirectOffsetOnAxis(
                                        ap=idx_t[:, m * 8:m * 8 + 1].bitcast(I32), axis=0))
                                nc.vector.tensor_tensor(
                                    x_res[:, m * C:(m + 1) * C], x_res[:, m * C:(m + 1) * C],
                                    q_sb[:, m * C:(m + 1) * C], op=ALU.add)
                    nc.vector.tensor_tensor(r2[:], r2[:], mv[:], op=ALU.subtract)
                    nc.vector.tensor_tensor(ql_acc[:], ql_acc[:], r2[:], op=ALU.add)
                    if lv < VL - 1:
                        with tc.tile_pool(name=f"pqt{li}_{lv}", bufs=3, space="PSUM") as pqt:
                            for k in range(KC):
                                ps_q = pqt.tile([128, TPC], BF16, name=f"psq{k}", tag="ps_qT")
                                for m in range(NT):
                                    nc.tensor.transpose(
                                        ps_q[:, m * 128:(m + 1) * 128],
                                        q_sb[:, m * C + k * 128: m * C + (k + 1) * 128],
                                        ident_bf16[:])
                                nc.vector.tensor_tensor(
                                    resT[:, k * TPC:(k + 1) * TPC],
                                    resT[:, k * TPC:(k + 1) * TPC], ps_q[:], op=ALU.subtract)

            layer_stack.close()

        for i in range(NT):
            nc.sync.dma_start(y_out[i * 128:(i + 1) * 128, :], x_res[:, i * C:(i + 1) * C])
        nc.sync.dma_start(ql_out[:], ql_acc[:])

    nc.compile()
    return nc


_CACHED = {}


def make_in_maps(consts_np, per_core_x, msels, layers):
    in_maps = []
    for c in range(NCORES):
        m = {"x_in": per_core_x[c], "msel_in": msels[c],
             "lsu_in": consts_np["lsu"], "lones_in": consts_np["lones"],
             "chsel_in": consts_np["chsel"]}
        for li, lay in enumerate(layers):
            for nm in ("mgW", "pW", "m1W", "m2W", "h1lo", "h1hi", "h2W"):
                m[f"L{li}_{nm}"] = lay[nm]
            for l in range(VL):
                m[f"L{li}_book{l}"] = lay["books"][l]
                if USE_GATHER_ADD:
                    m[f"L{li}_book32_{l}"] = lay["books32"][l]
                m[f"L{li}_e2t{l}"] = lay["e2t"][l]
                m[f"L{li}_negsq{l}"] = lay["negsq"][l]
        in_maps.append(m)
    return in_maps


def kernel(x, params):
    consts_np, per_core_x, msels, layers = host_prep(x, params)
    if "prog" not in _CACHED:
        _CACHED["prog"] = build_program(layers)
    nc = _CACHED["prog"]
    in_maps = make_in_maps(consts_np, per_core_x, msels, layers)
    res = run_bass_kernel_spmd(nc, in_maps, list(range(NCORES)))
    outs = res.results
    y = np.concatenate([outs[c]["y_out"] for c in range(NCORES)], 0).reshape(B, T, C)
    ql_total = float(sum(outs[c]["ql_out"].astype(np.float64).sum() for c in range(NCORES)))
    ql = np.float32(ql_total / (B * T * C) / VL)
    return y.astype(np.float32), ql, np.float32(ql)


# revision 24
# speedup vs baseline: 1.0207x; 1.0207x over previous
"""Trainium2 Bass kernel for nn_DiscretizedManifoldTransformer.

Self-contained: takes FULL inputs (x [2,2048,768] f32 + params list of 4 layer
dicts), shards tokens across 8 NeuronCores, runs a single SPMD Bass program
(4 transformer blocks: 2x chunked card-passing layers, MLP, residual VQ), and
returns (x_out, ql, el) matching the jax reference.

Sharding: token-parallel. B*T = 4096 tokens -> 512 tokens (= 2 seq chunks of
256) per core. The only cross-core dependency is the carry prefix-scan over
chunk sums in each card layer; handled with a tiny (16x768 bf16) AllGather
followed by a per-core selection matmul.

Layouts (per core):
  T-major packed tile: [128, NT*F]   token-tile i in cols [i*F,(i+1)*F)
  C-major packed tile: [128, KC*512] channel-chunk k in cols [k*512,(k+1)*512)
Residual x stays resident in SBUF (f32). Matmul inputs bf16 (host-cast
weights); PSUM f32. sigmoid(g) is computed as 0.5*(tanh(g/2)+1) with the 0.5
folded algebraically downstream (cumsum runs at 2x scale; the cards LayerNorm
absorbs it exactly via a 4x eps). rsqrt = exponent-bits seed + ACT exp + 2
Newton steps (single ACT table set for the whole kernel). ql/el use
mean((q-res)^2) = ||res_next||^2/n with ||res||^2 tracked by r2 -= max_score.
"""

import sys

for _p in ("/opt/trn_rl_repo",):
    if _p not in sys.path:
        sys.path.insert(0, _p)

from contextlib import ExitStack

import numpy as np
import ml_dtypes

import concourse.bass as bass
import concourse.bacc as bacc
import concourse.mybir as mybir
import concourse.tile as tile
from concourse.bass_utils import run_bass_kernel_spmd
from concourse.masks import make_identity

F32 = mybir.dt.float32
BF16 = mybir.dt.bfloat16
I32 = mybir.dt.int32
U32 = mybir.dt.uint32
AF = mybir.ActivationFunctionType
ALU = mybir.AluOpType
AXX = mybir.AxisListType.X

B, T, C, H, S = 2, 2048, 768, 12, 256
D, N_CHUNKS = C // H, T // S          # 64, 8
FF, VL, NCODES, NLAYERS = 4 * C, 4, 1024, 4
EPS = 1e-5
NCORES = 8
TPC = B * T // NCORES                 # 512 tokens/core
NT = TPC // 128                       # 4 token tiles
KC = C // 128                         # 6
KF = FF // 128                        # 24
CHUNKS_PC = TPC // S                  # 2
LN2 = float(np.log(2.0))

bf16 = ml_dtypes.bfloat16

# feature flags (HW bring-up bisects)
USE_GATHER_ADD = False  # compute_op=add on indirect DMA wedges TRN2 (NRT_EXEC_UNIT_UNRECOVERABLE)


class Prog:
    def __init__(self, nc, tc, stats, scratch, consts):
        self.nc = nc
        self.tc = tc
        self.stats = stats
        self.scratch = scratch
        self.c = consts

    # -- tiny-op helpers ---------------------------------------------------

    def rsqrt(self, out_ap, in_ap, tag):
        """out = 1/sqrt(in), f32, small tiles. bits-seed + exp + 2 Newton."""
        nc = self.nc
        P_, n = in_ap.shape[0], in_ap.shape[1]
        t0 = self.stats.tile([P_, n], F32, name=f"{tag}_t0", tag=f"{tag}_t0")
        t1 = self.stats.tile([P_, n], F32, name=f"{tag}_t1", tag=f"{tag}_t1")
        nc.vector.tensor_copy(t0[:], in_ap.bitcast(I32))
        nc.vector.tensor_scalar(out=t0[:], in0=t0[:], scalar1=LN2 * (2.0 ** -23),
                                scalar2=127.0 * LN2, op0=ALU.mult, op1=ALU.subtract)
        nc.scalar.activation(out_ap, t0[:], AF.Exp, scale=-0.5)
        for _ in range(2):
            nc.vector.tensor_tensor(t1[:], out_ap, out_ap, op=ALU.mult)
            nc.vector.tensor_tensor(t1[:], t1[:], in_ap, op=ALU.mult)
            nc.vector.tensor_scalar(out=t1[:], in0=t1[:], scalar1=-0.5, scalar2=1.5,
                                    op0=ALU.mult, op1=ALU.add)
            nc.vector.tensor_tensor(out_ap, out_ap, t1[:], op=ALU.mult)

    def ln_stats(self, ssum_ap, ssq_ap, nelem, tag, eps_scale=1.0, want_r2=False,
                 want_mur=False):
        """mu = ssum/nelem, var = ssq/nelem - mu^2, r = rsqrt(var + eps_scale*EPS),
        mur = mu*r. Optional r2 = nelem*var*r^2. Returns (mu, r, mur, r2)."""
        nc = self.nc
        P_, n = ssum_ap.shape[0], ssum_ap.shape[1]
        st = self.stats
        mu = st.tile([P_, n], F32, name=f"{tag}_mu", tag=f"{tag}_mu")
        var = st.tile([P_, n], F32, name=f"{tag}_var", tag=f"{tag}_var")
        vpe = st.tile([P_, n], F32, name=f"{tag}_vpe", tag=f"{tag}_vpe")
        r = st.tile([P_, n], F32, name=f"{tag}_r", tag=f"{tag}_r")
        mur = st.tile([P_, n], F32, name=f"{tag}_mur", tag=f"{tag}_mur")
        inv = 1.0 / float(nelem)
        nc.vector.tensor_scalar(out=mu[:], in0=ssum_ap, scalar1=inv, scalar2=None, op0=ALU.mult)
        nc.vector.tensor_scalar(out=var[:], in0=ssq_ap, scalar1=inv, scalar2=None, op0=ALU.mult)
        nc.vector.tensor_tensor(vpe[:], mu[:], mu[:], op=ALU.mult)
        nc.vector.tensor_tensor(var[:], var[:], vpe[:], op=ALU.subtract)
        nc.vector.tensor_scalar(out=vpe[:], in0=var[:], scalar1=float(eps_scale) * EPS,
                                scalar2=None, op0=ALU.add)
        self.rsqrt(r[:], vpe[:], f"{tag}_rs")
        if want_mur:
            nc.vector.tensor_tensor(mur[:], mu[:], r[:], op=ALU.mult)
        r2 = None
        if want_r2:
            r2 = st.tile([P_, n], F32, name=f"{tag}_r2", tag=f"{tag}_r2")
            nc.vector.tensor_tensor(r2[:], r[:], r[:], op=ALU.mult)
            nc.vector.tensor_tensor(r2[:], r2[:], var[:], op=ALU.mult)
            nc.vector.tensor_scalar(out=r2[:], in0=r2[:], scalar1=float(nelem),
                                    scalar2=None, op0=ALU.mult)
        return mu, r, mur, r2

    def sums_of(self, x_fn, tag):
        """(sum, sumsq) [128, NT] over C for token tiles x_fn(i) [128, C] (SBUF f32)."""
        nc = self.nc
        ssum = self.stats.tile([128, NT], F32, name=f"{tag}_ssum", tag=f"{tag}_ssum")
        ssq = self.stats.tile([128, NT], F32, name=f"{tag}_ssq", tag=f"{tag}_ssq")
        for i in range(NT):
            xi = x_fn(i)
            nc.vector.tensor_reduce(out=ssum[:, i:i + 1], in_=xi, op=ALU.add, axis=AXX)
            sc = self.scratch.tile([128, C], BF16, name=f"{tag}_sc{i}", tag="lnsq_scratch")
            nc.scalar.activation(sc[:], xi, AF.Square, accum_out=ssq[:, i:i + 1])
        return ssum, ssq

    def ln_apply(self, out_fn, x_fn, mu, r, out_dtype_note=None):
        """out_i = (x_i - mu_i) * r_i for NT tiles."""
        nc = self.nc
        for i in range(NT):
            nc.vector.tensor_scalar(out=out_fn(i), in0=x_fn(i),
                                    scalar1=mu[:, i:i + 1], scalar2=r[:, i:i + 1],
                                    op0=ALU.subtract, op1=ALU.mult)

    def transpose_to_cmajor(self, dst_packed, src_fn, psum_pool, dtype, tag):
        """dst [128, KC*TPC] bf16 C-major <- src_fn(i) [128, C] T-major tiles."""
        nc = self.nc
        ident = self.c["ident_f32"] if dtype == F32 else self.c["ident_bf16"]
        for k in range(KC):
            pt = psum_pool.tile([128, TPC], dtype, name=f"{tag}_pt{k}",
                                tag=f"tp_{'f' if dtype == F32 else 'b'}")
            for i in range(NT):
                nc.tensor.transpose(pt[:, i * 128:(i + 1) * 128],
                                    src_fn(i)[:, k * 128:(k + 1) * 128], ident[:])
            nc.any.tensor_copy(dst_packed[:, k * TPC:(k + 1) * TPC], pt[:])

    def formB(self, psum_tile, xT_packed, w_packed, m, cout, kc=KC, nmax=512):
        """psum[128 tok, cout] = sum_k xT_chunk(k,m).T @ w_chunk(k) ."""
        nc = self.nc
        for s0 in range(0, cout, nmax):
            w_ = min(nmax, cout - s0)
            for k in range(kc):
                nc.tensor.matmul(psum_tile[:, s0:s0 + w_],
                                 lhsT=xT_packed[:, k * TPC + m * 128: k * TPC + (m + 1) * 128],
                                 rhs=w_packed[:, k * cout + s0: k * cout + s0 + w_],
                                 start=(k == 0), stop=(k == kc - 1))


def host_prep(x, params):
    xf = np.ascontiguousarray(np.asarray(x, np.float32).reshape(B * T, C))
    per_core_x = [np.ascontiguousarray(xf[c * TPC:(c + 1) * TPC]) for c in range(NCORES)]

    lsu = np.triu(np.ones((128, 128), np.float32), 1).astype(bf16)   # [s', s]: s' < s
    lones = np.ones((128, 128), np.float32).astype(bf16)
    chsel = np.zeros((CHUNKS_PC, CHUNKS_PC * 128), np.float32)
    for ch in range(CHUNKS_PC):
        chsel[ch, ch * 128:(ch + 1) * 128] = 1.0
    chsel = chsel.astype(bf16)

    msels = []
    for c in range(NCORES):
        m = np.zeros((NCORES * CHUNKS_PC, CHUNKS_PC), np.float32)
        for j in range(CHUNKS_PC):
            fc = c * CHUNKS_PC + j
            b_, n_ = fc // N_CHUNKS, fc % N_CHUNKS
            for k in range(NCORES * CHUNKS_PC):
                if k // N_CHUNKS == b_ and k % N_CHUNKS < n_:
                    m[k, j] = 1.0
        msels.append(m.astype(bf16))

    layers = []
    bad = []
    for li, p in enumerate(params):
        g = {k: np.asarray(v) for k, v in p.items()}
        lay = {nm: np.ascontiguousarray(g[nm].astype(bf16))
               for nm in ("pW", "m1W", "m2W", "h2W")}
        lay["mgW"] = np.ascontiguousarray(
            np.concatenate([g["mW"], g["gW"]], axis=1).astype(bf16))
        h1 = g["h1W"].astype(np.float32)
        lay["h1lo"] = np.ascontiguousarray(np.vstack([h1[0:64], h1[0:64]]).astype(bf16))
        lay["h1hi"] = np.ascontiguousarray(np.vstack([h1[64:128], h1[64:128]]).astype(bf16))
        books = g["books"].astype(np.float32)
        lay["books"] = [np.ascontiguousarray(books[l].astype(bf16)) for l in range(VL)]
        lay["books32"] = [np.ascontiguousarray(books[l]) for l in range(VL)]
        lay["e2t"] = [np.ascontiguousarray((2.0 * books[l].T).astype(bf16)) for l in range(VL)]
        lay["negsq"] = [np.ascontiguousarray(
            (-(books[l] ** 2).sum(1))[None, :].astype(np.float32)) for l in range(VL)]
        lay["ha"] = float(np.asarray(g["ha"]))
        lay["ma"] = float(np.asarray(g["ma"]))
        for nm in ("mb", "gb", "h1b", "h2b", "pb", "m1b", "m2b", "carry_b", "card_b",
                   "lnW_b", "ln1_b", "ln2_b", "ln3_b"):
            if not np.allclose(g[nm], 0.0):
                bad.append((li, nm))
        for nm in ("carry_g", "card_g", "lnW_g", "ln1_g", "ln2_g", "ln3_g"):
            if not np.allclose(g[nm], 1.0):
                bad.append((li, nm))
        layers.append(lay)
    if bad:
        raise NotImplementedError(f"non-identity LN gains / biases: {bad}")
    return {"lsu": lsu, "lones": lones, "chsel": chsel}, per_core_x, msels, layers


def build_program(layers, repeat=1):
    nc = bacc.Bacc("TRN2", target_bir_lowering=False, debug=False,
                   num_devices=NCORES)

    x_in = nc.dram_tensor("x_in", [TPC, C], F32, kind="ExternalInput").ap()
    msel_in = nc.dram_tensor("msel_in", [NCORES * CHUNKS_PC, CHUNKS_PC], BF16,
                             kind="ExternalInput").ap()
    lsu_in = nc.dram_tensor("lsu_in", [128, 128], BF16, kind="ExternalInput").ap()
    lones_in = nc.dram_tensor("lones_in", [128, 128], BF16, kind="ExternalInput").ap()
    chsel_in = nc.dram_tensor("chsel_in", [CHUNKS_PC, CHUNKS_PC * 128], BF16,
                              kind="ExternalInput").ap()
    wd = {}
    for li in range(NLAYERS):
        for nm, shp in (("mgW", [C, 2 * C]), ("pW", [C, C]),
                        ("m1W", [C, FF]), ("m2W", [FF, C]),
                        ("h1lo", [2 * D, 2 * D]), ("h1hi", [2 * D, 2 * D]),
                        ("h2W", [2 * D, D])):
            wd[(li, nm)] = nc.dram_tensor(f"L{li}_{nm}", shp, BF16, kind="ExternalInput").ap()
        for l in range(VL):
            wd[(li, "book", l)] = nc.dram_tensor(f"L{li}_book{l}", [NCODES, C], BF16,
                                                 kind="ExternalInput").ap()
            if USE_GATHER_ADD:
                wd[(li, "book32", l)] = nc.dram_tensor(f"L{li}_book32_{l}", [NCODES, C], F32,
                                                       kind="ExternalInput").ap()
            wd[(li, "e2t", l)] = nc.dram_tensor(f"L{li}_e2t{l}", [C, NCODES], BF16,
                                                kind="ExternalInput").ap()
            wd[(li, "negsq", l)] = nc.dram_tensor(f"L{li}_negsq{l}", [1, NCODES], F32,
                                                  kind="ExternalInput").ap()
    y_out = nc.dram_tensor("y_out", [TPC, C], F32, kind="ExternalOutput").ap()
    ql_out = nc.dram_tensor("ql_out", [128, NT], F32, kind="ExternalOutput").ap()

    with tile.TileContext(nc) as tc, ExitStack() as top:
        const_pool = top.enter_context(tc.tile_pool(name="consts", bufs=1))
        resident = top.enter_context(tc.tile_pool(name="resident", bufs=1))
        stats = top.enter_context(tc.tile_pool(name="stats", bufs=1))
        scratch = top.enter_context(tc.tile_pool(name="scratch", bufs=3))
        dram_pool = top.enter_context(tc.tile_pool(name="drambounce", bufs=2, space="DRAM"))

        ident_f32 = const_pool.tile([128, 128], F32)
        ident_bf16 = const_pool.tile([128, 128], BF16)
        make_identity(nc, ident_f32[:])
        make_identity(nc, ident_bf16[:])
        lsu = const_pool.tile([128, 128], BF16)
        lones = const_pool.tile([128, 128], BF16)
        msel = const_pool.tile([NCORES * CHUNKS_PC, CHUNKS_PC], BF16)
        chsel = const_pool.tile([CHUNKS_PC, CHUNKS_PC * 128], BF16)
        nc.sync.dma_start(lsu[:], lsu_in[:])
        nc.sync.dma_start(lones[:], lones_in[:])
        nc.sync.dma_start(msel[:], msel_in[:])
        nc.sync.dma_start(chsel[:], chsel_in[:])

        x_res = resident.tile([128, NT * C], F32)
        ql_acc = resident.tile([128, NT], F32)
        r2 = resident.tile([128, NT], F32)

        consts = {"ident_f32": ident_f32, "ident_bf16": ident_bf16}
        pr = Prog(nc, tc, stats, scratch, consts)

        def xr(i):
            return x_res[:, i * C:(i + 1) * C]

        x3sum = None  # carried from MLP residual add into ln3

        for li_r in range(repeat * NLAYERS):
            li = li_r
            lay = layers[li_r % NLAYERS]
            if li_r % NLAYERS == 0:
                # (re)load residual + zero the ql accumulator; repeats>1 are
                # only used for marginal-time benchmarking
                nc.vector.memset(ql_acc[:], 0.0)
                for i in range(NT):
                    nc.sync.dma_start(x_res[:, i * C:(i + 1) * C],
                                      x_in[i * 128:(i + 1) * 128, :])
            layer_stack = ExitStack()
            pr.stats = layer_stack.enter_context(
                tc.tile_pool(name=f"st{li}", bufs=1))

            # ================= card deliberation (2 calls) =================
            with ExitStack() as st:
                wpool = st.enter_context(tc.tile_pool(name=f"cw{li}", bufs=1))
                mg_sb = wpool.tile([128, KC * 2 * C], BF16, name=f"mg{li}")
                pw_sb = wpool.tile([128, KC * C], BF16, name=f"pw{li}")
                h1lo_sb = wpool.tile([128, 2 * D], BF16, name=f"h1lo{li}")
                h1hi_sb = wpool.tile([128, 2 * D], BF16, name=f"h1hi{li}")
                h2_sb = wpool.tile([128, D], BF16, name=f"h2{li}")
                for k in range(KC):
                    nc.sync.dma_start(mg_sb[:, k * 2 * C:(k + 1) * 2 * C], wd[(li % NLAYERS, "mgW")][k * 128:(k + 1) * 128, :])
                    nc.sync.dma_start(pw_sb[:, k * C:(k + 1) * C], wd[(li % NLAYERS, "pW")][k * 128:(k + 1) * 128, :])
                nc.sync.dma_start(h1lo_sb[:], wd[(li % NLAYERS, "h1lo")][:])
                nc.sync.dma_start(h1hi_sb[:], wd[(li % NLAYERS, "h1hi")][:])
                nc.sync.dma_start(h2_sb[:], wd[(li % NLAYERS, "h2W")][:])

                apool = st.enter_context(tc.tile_pool(name=f"ca{li}", bufs=1))
                u_tile = apool.tile([128, NT * C], F32, name=f"u{li}")

                cur_src = x_res
                for call in range(2):
                    tg = f"{li}_{call}"
                    with ExitStack() as cs:
                        cpool = cs.enter_context(tc.tile_pool(name=f"c{tg}", bufs=1))

                        def cur(i, _s=cur_src):
                            return _s[:, i * C:(i + 1) * C]

                        # ---- t = ln1(cur); tT (C-major bf16)
                        ssum, ssq = pr.sums_of(cur, f"l1_{tg}")
                        mu, r, mur, _ = pr.ln_stats(ssum[:], ssq[:], C, f"l1s_{tg}")
                        t_f32 = cpool.tile([128, NT * C], F32, name=f"t{tg}")
                        pr.ln_apply(lambda i: t_f32[:, i * C:(i + 1) * C], cur, mu, r)
                        tT = cpool.tile([128, KC * TPC], BF16, name=f"tT{tg}")
                        with tc.tile_pool(name=f"ptp{tg}", bufs=2, space="PSUM") as pst:
                            pr.transpose_to_cmajor(tT[:], lambda i: t_f32[:, i * C:(i + 1) * C],
                                                   pst, F32, f"tT{tg}")

                        # ---- marks & gates -> gm (2x scale: (tanh+1)*marks)
                        gm = cpool.tile([128, NT * C], BF16, name=f"gm{tg}")
                        with tc.tile_pool(name=f"pmg{tg}", bufs=2, space="PSUM") as pmg:
                            for m in range(NT):
                                ps_mg = pmg.tile([128, 2 * C], F32, name=f"psmg{m}", tag="ps_mg")
                                pr.formB(ps_mg, tT[:], mg_sb[:], m, 2 * C)
                                gt_t = scratch.tile([128, C], BF16, name=f"gtt{m}", tag="gt_tanh")
                                nc.scalar.activation(gt_t[:], ps_mg[:, C:2 * C], AF.Tanh, scale=0.5)
                                nc.vector.scalar_tensor_tensor(
                                    out=gm[:, m * C:(m + 1) * C], in0=gt_t[:], scalar=1.0,
                                    in1=ps_mg[:, 0:C], op0=ALU.add, op1=ALU.mult)

                        # ---- chunk sums (2x scale) -> AllGather -> carry -> ncar
                        cs_sb = cpool.tile([32 * (CHUNKS_PC - 1) + 1, C], BF16, name=f"cssb{tg}")
                        with tc.tile_pool(name=f"pcs{tg}", bufs=1, space="PSUM") as pcs:
                            for ch in range(CHUNKS_PC):
                                ps_cs = pcs.tile([1, C], F32, name=f"pscs{ch}", tag=f"ps_cs{ch}")
                                for half in range(2):
                                    m = 2 * ch + half
                                    for s0 in range(0, C, 512):
                                        w_ = min(512, C - s0)
                                        nc.tensor.matmul(
                                            ps_cs[:, s0:s0 + w_], lhsT=lones[:, 0:1],
                                            rhs=gm[:, m * C + s0: m * C + s0 + w_],
                                            start=(half == 0), stop=(half == 1))
                                nc.scalar.copy(cs_sb[32 * ch:32 * ch + 1, :], ps_cs[:])

                        bounce_in = dram_pool.tile([CHUNKS_PC, C], BF16,
                                                   name=f"bin{tg}", tag="bounce_in")
                        bounce_out = dram_pool.tile([NCORES * CHUNKS_PC, C], BF16,
                                                    name=f"bout{tg}", tag="bounce_out")
                        for ch in range(CHUNKS_PC):
                            nc.sync.dma_start(bounce_in[ch:ch + 1, :],
                                              cs_sb[32 * ch:32 * ch + 1, :])
                        nc.gpsimd.collective_compute(
                            "AllGather", ALU.bypass, replica_groups=[list(range(NCORES))],
                            ins=[bounce_in[:].opt()], outs=[bounce_out[:].opt()])
                        gath = cpool.tile([NCORES * CHUNKS_PC, C], BF16, name=f"gath{tg}")
                        nc.sync.dma_start(gath[:], bounce_out[:])

                        ncar = cpool.tile([CHUNKS_PC, C], BF16, name=f"ncar{tg}")
                        with tc.tile_pool(name=f"pcy{tg}", bufs=1, space="PSUM") as pcy:
                            ps_cy = pcy.tile([CHUNKS_PC, C], F32, name="pscy", tag="ps_cy")
                            for s0 in range(0, C, 512):
                                w_ = min(512, C - s0)
                                nc.tensor.matmul(ps_cy[:, s0:s0 + w_], lhsT=msel[:],
                                                 rhs=gath[:, s0:s0 + w_], start=True, stop=True)
                            # exact carry = 0.5 * psum
                            carry_f = cpool.tile([CHUNKS_PC, C], F32, name=f"cyf{tg}")
                            nc.vector.tensor_scalar(out=carry_f[:], in0=ps_cy[:], scalar1=0.5,
                                                    scalar2=None, op0=ALU.mult)
                        csum = stats.tile([CHUNKS_PC, H], F32, name="cysum", tag=f"cy_s{tg}")
                        csq = stats.tile([CHUNKS_PC, H], F32, name="cysq", tag=f"cy_q{tg}")
                        sqt = scratch.tile([CHUNKS_PC, C], F32, name="cysq2", tag="cy_sq")
                        nc.vector.tensor_tensor(sqt[:], carry_f[:], carry_f[:], op=ALU.mult)
                        nc.vector.tensor_reduce(out=csum[:], in_=carry_f[:].rearrange(
                            "p (g s) -> p g s", s=D), op=ALU.add, axis=AXX)
                        nc.vector.tensor_reduce(out=csq[:], in_=sqt[:].rearrange(
                            "p (g s) -> p g s", s=D), op=ALU.add, axis=AXX)
                        cmu, cr, _, _ = pr.ln_stats(csum[:], csq[:], D, f"cy_{tg}")
                        # ncar2 = 2 * LN(carry)  (2x so it matches gm's 2x scale)
                        cr2 = stats.tile([CHUNKS_PC, H], F32, name="cr2", tag=f"cy_r2{tg}")
                        nc.vector.tensor_scalar(out=cr2[:], in0=cr[:], scalar1=2.0,
                                                scalar2=None, op0=ALU.mult)
                        cen = scratch.tile([CHUNKS_PC, C], F32, name="cycen", tag="cy_cen")
                        nc.vector.tensor_tensor(
                            cen[:].rearrange("p (g s) -> p g s", s=D),
                            carry_f[:].rearrange("p (g s) -> p g s", s=D),
                            cmu[:].broadcast_to([CHUNKS_PC, H, D]), op=ALU.subtract)
                        nc.vector.tensor_tensor(
                            ncar[:].rearrange("p (g s) -> p g s", s=D),
                            cen[:].rearrange("p (g s) -> p g s", s=D),
                            cr2[:].broadcast_to([CHUNKS_PC, H, D]), op=ALU.mult)

                        # ---- cards: cumsum matmuls issued before the gather
                        # lands (they only need gm); ncar row + LN_D after.
                        cards = cpool.tile([128, NT * C], BF16, name=f"cards{tg}")
                        with tc.tile_pool(name=f"pcd{tg}", bufs=2, space="PSUM") as pcd:
                            for ch in range(CHUNKS_PC):
                                ps_cds = []
                                for half in range(2):
                                    m = 2 * ch + half
                                    ps_cd = pcd.tile([128, C], F32, name=f"pscd{m}", tag=f"ps_cd{half}")
                                    for s0 in range(0, C, 512):
                                        w_ = min(512, C - s0)
                                        first = True
                                        if half == 1:
                                            nc.tensor.matmul(
                                                ps_cd[:, s0:s0 + w_], lhsT=lones[:],
                                                rhs=gm[:, (m - 1) * C + s0: (m - 1) * C + s0 + w_],
                                                start=True, stop=False)
                                            first = False
                                        nc.tensor.matmul(
                                            ps_cd[:, s0:s0 + w_], lhsT=lsu[:],
                                            rhs=gm[:, m * C + s0: m * C + s0 + w_],
                                            start=first, stop=False)
                                    ps_cds.append(ps_cd)
                                for half in range(2):
                                    m = 2 * ch + half
                                    ps_cd = ps_cds[half]
                                    for s0 in range(0, C, 512):
                                        w_ = min(512, C - s0)
                                        nc.tensor.matmul(
                                            ps_cd[:, s0:s0 + w_],
                                            lhsT=chsel[:, ch * 128:(ch + 1) * 128],
                                            rhs=ncar[:, s0:s0 + w_],
                                            start=False, stop=True)
                                    dsum = pr.stats.tile([128, H], F32, name=f"cdsum{m}", tag="cd_s")
                                    dsq = pr.stats.tile([128, H], F32, name=f"cdsq{m}", tag="cd_q")
                                    sq2 = scratch.tile([128, C], BF16, name=f"cdsq2{m}", tag="cd_sq")
                                    nc.scalar.activation(sq2[:], ps_cd[:], AF.Square)
                                    nc.vector.tensor_reduce(out=dsum[:], in_=ps_cd[:].rearrange(
                                        "p (g s) -> p g s", s=D), op=ALU.add, axis=AXX)
                                    nc.vector.tensor_reduce(out=dsq[:], in_=sq2[:].rearrange(
                                        "p (g s) -> p g s", s=D), op=ALU.add, axis=AXX)
                                    dmu, dr, _, _ = pr.ln_stats(dsum[:], dsq[:], D,
                                                                f"cd_{tg}_{m}", eps_scale=4.0)
                                    dr_bf = pr.stats.tile([128, H], BF16, name=f"drb{m}", tag="cd_rb")
                                    nc.vector.tensor_copy(dr_bf[:], dr[:])
                                    cen2 = scratch.tile([128, C], BF16, name=f"cdc{m}", tag="cd_cen")
                                    nc.vector.tensor_tensor(
                                        cen2[:].rearrange("p (g s) -> p g s", s=D),
                                        ps_cd[:].rearrange("p (g s) -> p g s", s=D),
                                        dmu[:].broadcast_to([128, H, D]), op=ALU.subtract)
                                    nc.vector.tensor_tensor(
                                        cards[:, m * C:(m + 1) * C].rearrange("p (g s) -> p g s", s=D),
                                        cen2[:].rearrange("p (g s) -> p g s", s=D),
                                        dr_bf[:].broadcast_to([128, H, D]), op=ALU.mult)

                        cardsT = cpool.tile([128, KC * TPC], BF16, name=f"cdT{tg}")
                        with tc.tile_pool(name=f"ptc{tg}", bufs=2, space="PSUM") as pst:
                            pr.transpose_to_cmajor(cardsT[:], lambda i: cards[:, i * C:(i + 1) * C],
                                                   pst, BF16, f"cdT{tg}")

                        # ---- per-head h1 + FEGE + h2 -> hoT (C-major bf16)
                        hoT = cpool.tile([128, KC * TPC], BF16, name=f"hoT{tg}")
                        ha = lay["ha"]
                        with tc.tile_pool(name=f"phh{tg}", bufs=3, space="PSUM") as phh:
                            for h in range(H):
                                k, rowh = divmod(h, 2)
                                rsl = slice(rowh * 64, rowh * 64 + 64)
                                ps_h = phh.tile([128, TPC], F32, name=f"psh{h}", tag="ps_h1")
                                nc.tensor.matmul(ps_h[:], lhsT=h1lo_sb[rsl, :],
                                                 rhs=tT[:, k * TPC:(k + 1) * TPC][rsl, :],
                                                 start=True, stop=False)
                                nc.tensor.matmul(ps_h[:], lhsT=h1hi_sb[rsl, :],
                                                 rhs=cardsT[:, k * TPC:(k + 1) * TPC][rsl, :],
                                                 start=False, stop=True)
                                sqh = scratch.tile([128, TPC], BF16, name=f"sqh{h}", tag="fg_sq")
                                nc.scalar.activation(sqh[:], ps_h[:], AF.Square)
                                eh = scratch.tile([128, TPC], BF16, name=f"eh{h}", tag="fg_e")
                                nc.scalar.activation(eh[:], sqh[:], AF.Exp, scale=-0.5)
                                nc.vector.tensor_scalar(out=eh[:], in0=eh[:], scalar1=ha,
                                                        scalar2=1.0, op0=ALU.mult, op1=ALU.add)
                                hf = scratch.tile([128, TPC], BF16, name=f"hf{h}", tag="fg_hf")
                                nc.vector.tensor_tensor(hf[:], ps_h[:], eh[:], op=ALU.mult)
                                ps_o = phh.tile([128, TPC], F32, name=f"pso{h}", tag="ps_h2",
                                                bufs=2) if rowh == 0 else ps_o_prev
                                tp = (0, 64) if rowh == 1 else None
                                nc.tensor.matmul(ps_o[rsl, :], lhsT=h2_sb[:], rhs=hf[:],
                                                 start=True, stop=True, tile_position=tp)
                                if rowh == 1:
                                    nc.any.tensor_copy(hoT[:, k * TPC:(k + 1) * TPC], ps_o[:])
                                ps_o_prev = ps_o

                        # ---- pW + lnW + residual: dst = x + t + LN(o)
                        dst = u_tile if call == 0 else x_res
                        o_f32 = cpool.tile([128, NT * C], F32, name=f"of{tg}")
                        osum = stats.tile([128, NT], F32, name="osum", tag=f"lnw_s{tg}")
                        osq = stats.tile([128, NT], F32, name="osq", tag=f"lnw_q{tg}")
                        with tc.tile_pool(name=f"ppw{tg}", bufs=3, space="PSUM") as ppw:
                            for m in range(NT):
                                ps_p = ppw.tile([128, C], F32, name=f"psp{m}", tag="ps_pw")
                                pr.formB(ps_p, hoT[:], pw_sb[:], m, C)
                                nc.scalar.activation(o_f32[:, m * C:(m + 1) * C], ps_p[:],
                                                     AF.Copy, accum_out=osum[:, m:m + 1])
                                junk = scratch.tile([128, C], BF16, name=f"jk{m}", tag="lnw_junk")
                                nc.scalar.activation(junk[:], ps_p[:], AF.Square,
                                                     accum_out=osq[:, m:m + 1])
                        omu, orr, omur, _ = pr.ln_stats(osum[:], osq[:], C, f"lnw_{tg}", want_mur=True)
                        for m in range(NT):
                            xa = scratch.tile([128, C], F32, name=f"xa{m}", tag="xa")
                            # w = o*r + t ; dst = (w - mu*r) + x
                            nc.vector.scalar_tensor_tensor(
                                out=xa[:], in0=o_f32[:, m * C:(m + 1) * C],
                                scalar=orr[:, m:m + 1], in1=t_f32[:, m * C:(m + 1) * C],
                                op0=ALU.mult, op1=ALU.add)
                            nc.vector.scalar_tensor_tensor(
                                out=dst[:, m * C:(m + 1) * C], in0=xa[:],
                                scalar=omur[:, m:m + 1], in1=x_res[:, m * C:(m + 1) * C],
                                op0=ALU.subtract, op1=ALU.add)
                    cur_src = u_tile

            # ================= MLP =================
            with ExitStack() as st:
                wpool = st.enter_context(tc.tile_pool(name=f"mw{li}", bufs=1))
                m1_sb = wpool.tile([128, KC * FF], BF16, name=f"m1{li}")
                m2_sb = wpool.tile([128, KF * C], BF16, name=f"m2{li}")
                for k in range(KC):
                    nc.sync.dma_start(m1_sb[:, k * FF:(k + 1) * FF],
                                      wd[(li % NLAYERS, "m1W")][k * 128:(k + 1) * 128, :])
                for k in range(KF):
                    nc.sync.dma_start(m2_sb[:, k * C:(k + 1) * C],
                                      wd[(li % NLAYERS, "m2W")][k * 128:(k + 1) * 128, :])
                mpool = st.enter_context(tc.tile_pool(name=f"ma{li}", bufs=1))

                ssum, ssq = pr.sums_of(xr, f"l2_{li}")
                mu, r, _, _ = pr.ln_stats(ssum[:], ssq[:], C, f"l2s_{li}")
                xn2 = mpool.tile([128, NT * C], BF16, name=f"xn2_{li}")
                pr.ln_apply(lambda i: xn2[:, i * C:(i + 1) * C], xr, mu, r)
                xn2T = mpool.tile([128, KC * TPC], BF16, name=f"xn2T_{li}")
                with tc.tile_pool(name=f"ptm{li}", bufs=2, space="PSUM") as pst:
                    pr.transpose_to_cmajor(xn2T[:], lambda i: xn2[:, i * C:(i + 1) * C],
                                           pst, BF16, f"xn2T{li}")

                hmlp = mpool.tile([128, KF * TPC], BF16, name=f"hm_{li}")
                ma = lay["ma"]
                with tc.tile_pool(name=f"pm1{li}", bufs=4, space="PSUM") as pm1:
                    for mo in range(KF):
                        ps_m = pm1.tile([128, TPC], F32, name=f"psm{mo}", tag="ps_m1")
                        for k in range(KC):
                            nc.tensor.matmul(
                                ps_m[:], lhsT=m1_sb[:, k * FF + mo * 128: k * FF + (mo + 1) * 128],
                                rhs=xn2T[:, k * TPC:(k + 1) * TPC],
                                start=(k == 0), stop=(k == KC - 1))
                        sqh = scratch.tile([128, TPC], BF16, name=f"msq{mo}", tag="fg_sq")
                        nc.scalar.activation(sqh[:], ps_m[:], AF.Square)
                        eh = scratch.tile([128, TPC], BF16, name=f"me{mo}", tag="fg_e")
                        nc.scalar.activation(eh[:], sqh[:], AF.Exp, scale=-0.5)
                        nc.vector.tensor_scalar(out=eh[:], in0=eh[:], scalar1=ma,
                                                scalar2=1.0, op0=ALU.mult, op1=ALU.add)
                        nc.vector.tensor_tensor(hmlp[:, mo * TPC:(mo + 1) * TPC], ps_m[:],
                                                eh[:], op=ALU.mult)

                x3sum = stats.tile([128, NT], F32, name="x3sum", tag=f"x3_s{li}")
                with tc.tile_pool(name=f"pm2{li}", bufs=2, space="PSUM") as pm2:
                    for m in range(NT):
                        ps_o = pm2.tile([128, C], F32, name=f"psmo{m}", tag="ps_m2")
                        pr.formB(ps_o, hmlp[:], m2_sb[:], m, C, kc=KF)
                        nc.vector.scalar_tensor_tensor(
                            out=x_res[:, m * C:(m + 1) * C], in0=ps_o[:], scalar=1.0,
                            in1=x_res[:, m * C:(m + 1) * C], op0=ALU.mult, op1=ALU.add,
                            accum_out=x3sum[:, m:m + 1])

            # ================= RVQ =================
            with ExitStack() as st:
                rpool = st.enter_context(tc.tile_pool(name=f"rv{li}", bufs=1))
                epool = st.enter_context(tc.tile_pool(name=f"re{li}", bufs=2))

                ssq3 = stats.tile([128, NT], F32, name="ssq3", tag=f"x3_q{li}")
                for i in range(NT):
                    sc = scratch.tile([128, C], BF16, name=f"x3sc{i}", tag="lnsq_scratch")
                    nc.scalar.activation(sc[:], xr(i), AF.Square, accum_out=ssq3[:, i:i + 1])
                mu3, r3, _, r2_0 = pr.ln_stats(x3sum[:], ssq3[:], C, f"l3s_{li}", want_r2=True)
                nc.vector.tensor_copy(r2[:], r2_0[:])
                xn3 = rpool.tile([128, NT * C], BF16, name=f"xn3_{li}")
                pr.ln_apply(lambda i: xn3[:, i * C:(i + 1) * C], xr, mu3, r3)
                resT = rpool.tile([128, KC * TPC], BF16, name=f"resT_{li}")
                with tc.tile_pool(name=f"ptr{li}", bufs=2, space="PSUM") as pst:
                    pr.transpose_to_cmajor(resT[:], lambda i: xn3[:, i * C:(i + 1) * C],
                                           pst, BF16, f"resT{li}")

                mv = stats.tile([128, NT], F32, name="mv", tag=f"mv{li}")
                for lv in range(VL):
                    e2t_sb = epool.tile([128, KC * NCODES], BF16, name=f"e2t{lv}", tag="e2t")
                    for k in range(KC):
                        nc.sync.dma_start(e2t_sb[:, k * NCODES:(k + 1) * NCODES],
                                          wd[(li % NLAYERS, "e2t", lv)][k * 128:(k + 1) * 128, :])
                    nsq_sb = epool.tile([1, NCODES], F32, name=f"nsq{lv}", tag="nsq")
                    nc.sync.dma_start(nsq_sb[:], wd[(li % NLAYERS, "negsq", lv)][:])
                    nsq_bf = epool.tile([1, NCODES], BF16, name=f"nsqb{lv}", tag="nsqb")
                    nc.vector.tensor_copy(nsq_bf[:], nsq_sb[:])

                    idx_t = rpool.tile([128, NT * 8], U32, name=f"idx{lv}", tag="idx")
                    q_sb = rpool.tile([128, NT * C], BF16, name=f"q{lv}", tag="q_sb")
                    with tc.tile_pool(name=f"psc{li}_{lv}", bufs=3, space="PSUM") as psc:
                        for m in range(NT):
                            ps_s = psc.tile([128, NCODES], F32, name=f"pss{m}", tag="ps_sc")
                            for s0 in range(0, NCODES, 512):
                                for k in range(KC):
                                    nc.tensor.matmul(
                                        ps_s[:, s0:s0 + 512],
                                        lhsT=resT[:, k * TPC + m * 128: k * TPC + (m + 1) * 128],
                                        rhs=e2t_sb[:, k * NCODES + s0: k * NCODES + s0 + 512],
                                        start=(k == 0), stop=False)
                                nc.tensor.matmul(ps_s[:, s0:s0 + 512], lhsT=lones[0:1, :],
                                                 rhs=nsq_bf[:, s0:s0 + 512],
                                                 start=False, stop=True)
                            mx8 = stats.tile([128, 8], F32, name=f"mx8_{m}", tag=f"mx8_{m}")
                            nc.vector.max(mx8[:], ps_s[:])
                            nc.vector.tensor_copy(mv[:, m:m + 1], mx8[:, 0:1])
                            nc.vector.max_index(idx_t[:, m * 8:(m + 1) * 8], mx8[:], ps_s[:])
                            if USE_GATHER_ADD:
                                nc.gpsimd.indirect_dma_start(
                                    out=x_res[:, m * C:(m + 1) * C], out_offset=None,
                                    in_=wd[(li % NLAYERS, "book32", lv)][:],
                                    in_offset=bass.IndirectOffsetOnAxis(
                                        ap=idx_t[:, m * 8:m * 8 + 1].bitcast(I32), axis=0),
                                    compute_op=ALU.add)
                                if lv < VL - 1:
                                    nc.gpsimd.indirect_dma_start(
                                        out=q_sb[:, m * C:(m + 1) * C], out_offset=None,
                                        in_=wd[(li % NLAYERS, "book", lv)][:],
                                        in_offset=bass.IndirectOffsetOnAxis(
                                            ap=idx_t[:, m * 8:m * 8 + 1].bitcast(I32), axis=0))
                            else:
                                nc.gpsimd.indirect_dma_start(
                                    out=q_sb[:, m * C:(m + 1) * C], out_offset=None,
                                    in_=wd[(li % NLAYERS, "book", lv)][:],
                                    in_offset=bass.IndirectOffsetOnAxis(
                                        ap=idx_t[:, m * 8:m * 8 + 1].bitcast(I32), axis=0))
                                nc.vector.tensor_tensor(
                                    x_res[:, m * C:(m + 1) * C], x_res[:, m * C:(m + 1) * C],
                                    q_sb[:, m * C:(m + 1) * C], op=ALU.add)
                    nc.vector.tensor_tensor(r2[:], r2[:], mv[:], op=ALU.subtract)
                    nc.vector.tensor_tensor(ql_acc[:], ql_acc[:], r2[:], op=ALU.add)
                    if lv < VL - 1:
                        with tc.tile_pool(name=f"pqt{li}_{lv}", bufs=3, space="PSUM") as pqt:
                            for k in range(KC):
                                ps_q = pqt.tile([128, TPC], BF16, name=f"psq{k}", tag="ps_qT")
                                for m in range(NT):
                                    nc.tensor.transpose(
                                        ps_q[:, m * 128:(m + 1) * 128],
                                        q_sb[:, m * C + k * 128: m * C + (k + 1) * 128],
                                        ident_bf16[:])
                                nc.vector.tensor_tensor(
                                    resT[:, k * TPC:(k + 1) * TPC],
                                    resT[:, k * TPC:(k + 1) * TPC], ps_q[:], op=ALU.subtract)

            layer_stack.close()

        for i in range(NT):
            nc.sync.dma_start(y_out[i * 128:(i + 1) * 128, :], x_res[:, i * C:(i + 1) * C])
        nc.sync.dma_start(ql_out[:], ql_acc[:])

    nc.compile()
    return nc


_CACHED = {}


def make_in_maps(consts_np, per_core_x, msels, layers):
    in_maps = []
    for c in range(NCORES):
        m = {"x_in": per_core_x[c], "msel_in": msels[c],
             "lsu_in": consts_np["lsu"], "lones_in": consts_np["lones"],
             "chsel_in": consts_np["chsel"]}
        for li, lay in enumerate(layers):
            for nm in ("mgW", "pW", "m1W", "m2W", "h1lo", "h1hi", "h2W"):
                m[f"L{li}_{nm}"] = lay[nm]
            for l in range(VL):
                m[f"L{li}_book{l}"] = lay["books"][l]
                if USE_GATHER_ADD:
                    m[f"L{li}_book32_{l}"] = lay["books32"][l]
                m[f"L{li}_e2t{l}"] = lay["e2t"][l]
                m[f"L{li}_negsq{l}"] = lay["negsq"][l]
        in_maps.append(m)
    return in_maps


def kernel(x, params):
    consts_np, per_core_x, msels, layers = host_prep(x, params)
    if "prog" not in _CACHED:
        _CACHED["prog"] = build_program(layers)
    nc = _CACHED["prog"]
    in_maps = make_in_maps(consts_np, per_core_x, msels, layers)
    res = run_bass_kernel_spmd(nc, in_maps, list(range(NCORES)))
    outs = res.results
    y = np.concatenate([outs[c]["y_out"] for c in range(NCORES)], 0).reshape(B, T, C)
    ql_total = float(sum(outs[c]["ql_out"].astype(np.float64).sum() for c in range(NCORES)))
    ql = np.float32(ql_total / (B * T * C) / VL)
    return y.astype(np.float32), ql, np.float32(ql)
